# revision 41
# baseline (speedup 1.0000x reference)
"""MoE layer (E=8 experts, top-2 routing) on 8 Trainium2 NeuronCores.

Expert-parallel sharding: core e holds expert e's weights (w1/w2/b1).
Tokens are dispatched (host-side router, exact fp32) to the cores of their
top-2 experts; each core runs its expert's FFN on its gathered tokens in
bf16 (h = relu(x@w1+b1); out = h@w2). The bias b2 and the combine weights
are folded into the host-side scatter-add return, keeping the device
critical path to pure matmul work.

Shapes (hardcoded per the problem spec):
  x [2, 2048, 512] f32, router_w [8, 512], w1_all [8, 2048, 512],
  b1_all [8, 2048], w2_all [8, 512, 2048], b2_all [8, 512].

Per-core device program (bf16 operands, fp32 PSUM):
  mm1: h[dff, tok] = relu(w1.T-tiles @ xgT + b1)   (4 k-steps of 128)
  mm2: out[tok, d] = h-tiles.T @ w2-tiles          (16 m-steps of 128)
PE floor = cap*128 cycles = 61.4us at cap=1152; everything else (DMA
streaming, relu on DVE, PSUM->SBUF copies, output DMA) overlaps. A PE
warmup burst on a memset tile runs during the DMA head so the HAM cold
throttle (1.2GHz until ~3.5us of sustained PE activity) lifts before the
real matmuls begin.
"""

import sys

sys.path.insert(0, "/opt/trn_rl_repo")

import numpy as np
import ml_dtypes

import concourse.mybir as mybir
import concourse.tile as tile
from concourse import bacc

D_MODEL = 512
DFF = 2048
E = 8
K = 2
L = 2 * 2048  # total tokens
N_CORES = 8

FP = mybir.dt.float32
BF = mybir.dt.bfloat16
NPBF = ml_dtypes.bfloat16

# Per-expert token capacity (padded). Expected load is L*K/E = 1024 with
# std ~30 under the near-uniform router; seed-0 max count is 1092. The
# program is built for the actual max count (rounded to a multiple of 4),
# so this is only the floor. PE cost is cap*128 cycles, so every padded
# column costs 53ns — no rounding to 128.
CAP = 1092

KD = D_MODEL // 128  # 4 k-slices for mm1
MD = DFF // 128  # 16 dff tiles

_PROG_CACHE: dict = {}


def build_program(cap: int, warm: int = 9, bs: int = 512):
    """One SPMD program, run on all 8 cores; per-core data selects the expert.

    Per-core inputs (bf16 except b1):
      xgT  [512, cap]   gathered tokens for this expert, transposed (d-major)
      w1t  [512, 2048]  w1_e.T
      w2m  [512, 2048]  w2_e.T tiles merged: row J*128+p, col jj*512+d =
                        w2_e.T[(J*4+jj)*128+p, d]
      b1r  [16, 128, 1] b1_e (fp32)
    Output:
      out  [cap, 512]   bf16 expert FFN outputs (no bias/scale), row s =
                        gathered token slot s
    """
    nc = bacc.Bacc("TRN2", target_bir_lowering=False, debug=False)

    xgT = nc.dram_tensor("xgT", [D_MODEL, cap], BF, kind="ExternalInput")
    w1t = nc.dram_tensor("w1t", [D_MODEL, DFF], BF, kind="ExternalInput")
    w2m = nc.dram_tensor("w2m", [D_MODEL, DFF], BF, kind="ExternalInput")
    b1r = nc.dram_tensor("b1r", [DFF // 128, 128, 1], FP, kind="ExternalInput")
    # Full 128-token tiles go to `out` rows; a sub-128 remainder tile is
    # computed transposed (moving dim = remainder, 4x cheaper for 68 toks)
    # and lands in `outt` [d, r].
    full_cols = (cap // 128) * 128
    rem = cap - full_cols
    out = nc.dram_tensor("out", [full_cols, D_MODEL], BF, kind="ExternalOutput")
    # outt keeps the SBUF staging layout [p, dj, tok] so the final DMA is
    # one contiguous transfer; the host untangles it (d = dj*128+p).
    outt = (
        nc.dram_tensor("outt", [128, KD, rem], BF, kind="ExternalOutput")
        if rem
        else None
    )

    # token blocks: big early blocks so w1 streaming keeps up with the
    # m-loop's weight consumption; small last block to shrink the tail.
    blocks = []
    off = 0
    while off < cap:
        w = min(bs, cap - off)
        blocks.append((off, w))
        off += w

    with tile.TileContext(nc) as tc:
        with (
            tc.tile_pool(name="weights", bufs=1) as wpool,
            tc.tile_pool(name="h", bufs=4) as hpool,
            tc.tile_pool(name="psum", bufs=4, space="PSUM") as ppool,
            tc.tile_pool(name="outp", bufs=3) as opool,
            tc.tile_pool(name="consts", bufs=1) as cpool,
        ):
            xgT_sb = [
                wpool.tile([128, cap], BF, tag=f"xgT{k}", name=f"xgT_sb{k}")
                for k in range(KD)
            ]
            w1t_sb = [
                wpool.tile([128, DFF], BF, tag=f"w1t{k}", name=f"w1t_sb{k}")
                for k in range(KD)
            ]
            w2m_sb = [
                wpool.tile([128, DFF], BF, tag=f"w2m{j}", name=f"w2m_sb{j}")
                for j in range(KD)
            ]
            b1_sb = wpool.tile([128, MD], FP, tag="b1")
            warm_sb = cpool.tile([128, 512], BF, tag="warm")

            # --- DMA issue schedule ---
            # Only scalar (ACT), sync (SP) and gpsimd can issue DMAs; each
            # engine feeds its own hardware queue (~0.6us per issue, queues
            # ramp to only ~100-150 GB/s each early on). w1 is the early
            # critical stream: mm1's m-loop eats 128KB per 853ns at full
            # speed, so w1 gets strict priority and fine chunks. gpsimd
            # drops out after the w1 waves (it runs half the relu stream);
            # sync is the pure-DMA mule; scalar finishes early then owns
            # the output DMAs.
            nc.gpsimd.memset(warm_sb[:], 0.0)
            b0 = blocks[0][1]

            def xgT_dma(eng, k, lo, hi):
                eng.dma_start(
                    out=xgT_sb[k][:, lo:hi], in_=xgT[k * 128 : (k + 1) * 128, lo:hi]
                )

            def w1_dma(eng, k, lo, hi):
                eng.dma_start(
                    out=w1t_sb[k][:, lo:hi], in_=w1t[k * 128 : (k + 1) * 128, lo:hi]
                )

            # All input transfers in consumption order, strict round-robin
            # across the three issue queues so each queue's byte stream is
            # balanced and roughly need-ordered. scalar's queue head also
            # carries the ACT table load (~1.3us, for the relu stream), so
            # the rotation starts on sync/gpsimd.
            rr = [nc.sync, nc.gpsimd, nc.scalar]
            ri = 0

            def issue(dst, src):
                nonlocal ri
                rr[ri % 3].dma_start(out=dst, in_=src)
                ri += 1

            # Front order = need order, with descriptor COUNT kept minimal
            # on the early queues (each issue costs ~0.62us of queue time;
            # v7's finer 256-col split regressed 12us by delaying w1).
            for k in range(KD):
                issue(xgT_sb[k][:, :512], xgT[k * 128 : (k + 1) * 128, :512])
            for k in range(KD):
                issue(w1t_sb[k][:, :128], w1t[k * 128 : (k + 1) * 128, :128])
            # block-1 tokens early: mm1 interleaves blocks 0/1 so each w1
            # m-tile serves two m-groups back-to-back (halves w1 demand)
            for k in range(KD):
                issue(
                    xgT_sb[k][:, 512:1024], xgT[k * 128 : (k + 1) * 128, 512:1024]
                )
            for lo, hi in [(128, 384), (384, 1024), (1024, 2048)]:
                for k in range(KD):
                    issue(
                        w1t_sb[k][:, lo:hi], w1t[k * 128 : (k + 1) * 128, lo:hi]
                    )
                if lo == 128:
                    nc.gpsimd.dma_start(
                        out=b1_sb[:], in_=b1r.rearrange("m p o -> p (m o)")
                    )
            # w2 (needed from the mm2 phase, ~38us in) and the last block.
            for j in range(KD):
                issue(w2m_sb[j][:], w2m[j * 128 : (j + 1) * 128, :])
            for k in range(KD):
                issue(
                    xgT_sb[k][:, 1024:cap], xgT[k * 128 : (k + 1) * 128, 1024:cap]
                )

            # --- PE warmup: lift the HAM cold throttle during the DMA head ---
            if warm:
                ps_w = ppool.tile([128, 512], FP, tag="ps2", bufs=4, name="ps_warm")
                for wi in range(warm):
                    nc.tensor.matmul(
                        ps_w[:], warm_sb[:, :128], warm_sb[:],
                        start=True, stop=True,
                    )
                warm_sink = cpool.tile([1, 8], FP, tag="warm_sink")
                nc.vector.tensor_copy(warm_sink[:], ps_w[0:1, 0:8])

            # --- main compute ---
            h_sb = {}

            def emit_mm1(bi, off, ncols, m):
                """mm1 m-group: ps = w1[:,m] block-matmul, then relu+bias
                into a bf16 h tile. relu split DVE/ACT (Pool can't read
                PSUM; a 512-col tile costs ~750ns on DVE / ~500ns on ACT
                vs mm1's 853ns per-m period)."""
                ps = ppool.tile([128, ncols], FP, tag="ps1", bufs=3, name="ps")
                for k in range(KD):
                    nc.tensor.matmul(
                        ps[:],
                        w1t_sb[k][:, m * 128 : (m + 1) * 128],
                        xgT_sb[k][:, off : off + ncols],
                        start=(k == 0),
                        stop=(k == KD - 1),
                    )
                h = hpool.tile([128, ncols], BF, tag=f"h{m}", name=f"h_{m}")
                # block 0's relus feed the interleaved mm2 partials, so
                # they get the dedicated DVE stream; other blocks use ACT
                # (with its ~1.3us one-time table load at queue head).
                if bi != 0:
                    nc.scalar.activation(
                        h[:],
                        ps[:],
                        mybir.ActivationFunctionType.Relu,
                        bias=b1_sb[:, m : m + 1],
                    )
                else:
                    nc.vector.tensor_scalar(
                        h[:],
                        ps[:],
                        b1_sb[:, m : m + 1],
                        0.0,
                        mybir.AluOpType.add,
                        mybir.AluOpType.max,
                    )
                h_sb[(bi, m)] = h

            def emit_mm2(bi, off, t, d_split=False):
                """mm2 for one 128-token tile + bf16 cast + output DMA.

                d_split runs two 256-wide PSUM groups (same total PE cost;
                128-col pacing shows LDWEIGHTS fully overlaps) so the first
                half's cast+DMA overlaps the second half's matmuls — used
                for the very last tile to shorten the tail."""
                u = off // 128 + t
                halves = ((0, 256), (256, 512)) if d_split else ((0, 512),)
                o = opool.tile([128, D_MODEL], BF, tag="o")
                for dlo, dhi in halves:
                    ps2 = ppool.tile(
                        [128, D_MODEL], FP, tag="ps2", bufs=4, name="ps2"
                    )
                    for m in range(MD):
                        nc.tensor.matmul(
                            ps2[:, : dhi - dlo],
                            h_sb[(bi, m)][:, t * 128 : (t + 1) * 128],
                            w2m_sb[m // 4][
                                :, (m % 4) * 512 + dlo : (m % 4) * 512 + dhi
                            ],
                            start=(m == 0),
                            stop=(m == MD - 1),
                        )
                    nc.vector.tensor_copy(o[:, dlo:dhi], ps2[:, : dhi - dlo])
                    nc.scalar.dma_start(
                        out=out[u * 128 : (u + 1) * 128, dlo:dhi],
                        in_=o[:, dlo:dhi],
                    )

            # Phase A: mm1 interleaved over blocks 0/1 with block 0's mm2
            # accumulated in open PSUM groups one m-step behind. Each
            # m-step is mm1(b0)+mm1(b1)+4 mm2 partials = 2.56us while
            # consuming one w1 m-tile (50GB/s) and one w2 m-tile — the
            # DMA queues ramp comfortably ahead of this, which kills the
            # mid-phase stalls that reset the HAM clock.
            ps2_open = [
                ppool.tile([128, D_MODEL], FP, tag="ps2", bufs=4, name=f"ps2o{t}")
                for t in range(4)
            ]
            for m in range(MD + 1):
                if m < MD:
                    emit_mm1(0, 0, 512, m)
                    emit_mm1(1, 512, 512, m)
                if m > 0:
                    pm = m - 1
                    for t in range(4):
                        nc.tensor.matmul(
                            ps2_open[t][:],
                            h_sb[(0, pm)][:, t * 128 : (t + 1) * 128],
                            w2m_sb[pm // 4][
                                :, (pm % 4) * 512 : (pm % 4 + 1) * 512
                            ],
                            start=(pm == 0),
                            stop=(pm == MD - 1),
                        )
            for t in range(4):
                o = opool.tile([128, D_MODEL], BF, tag="o")
                nc.vector.tensor_copy(o[:], ps2_open[t][:])
                nc.scalar.dma_start(out=out[t * 128 : (t + 1) * 128, :], in_=o[:])
            # Phase B: mm2 over block 1 (w2 is fully resident by now).
            for t in range(4):
                emit_mm2(1, 512, t)
            # Phase C: the tail block(s) past the first 1024 tokens. The
            # final sub-128 remainder tile (if any) runs mm2 transposed:
            # out[d, tok] with the token remainder as the moving dim —
            # 68 tokens cost 4x16x68 cycles instead of 16x512 — and one
            # small rearranged DMA. Otherwise the last full tile's mm2 is
            # d-split to shorten the tail chain.
            bi = 4
            tail_blocks = list(range(1024, cap, 512))
            for off in tail_blocks:
                w = min(512, cap - off)
                for m in range(MD):
                    emit_mm1(bi, off, w, m)
                full_t = w // 128
                for t in range(full_t):
                    last = off == tail_blocks[-1] and t == full_t - 1 and not (
                        off + w == cap and cap % 128
                    )
                    emit_mm2(bi, off, t, d_split=last)
                r = w % 128
                if off + w == cap and r:
                    oT = opool.tile([128, KD, r], BF, tag="oT", bufs=1)
                    for dj in range(KD):
                        psT = ppool.tile(
                            [128, r], FP, tag="ps2", bufs=4, name="psT"
                        )
                        for m in range(MD):
                            nc.tensor.matmul(
                                psT[:],
                                w2m_sb[m // 4][
                                    :,
                                    (m % 4) * 512
                                    + dj * 128 : (m % 4) * 512
                                    + (dj + 1) * 128,
                                ],
                                h_sb[(bi, m)][:, full_t * 128 : full_t * 128 + r],
                                start=(m == 0),
                                stop=(m == MD - 1),
                            )
                        nc.vector.tensor_copy(oT[:, dj, :], psT[:])
                    # one contiguous DMA for all four d-tiles (separate
                    # issues cost ~0.6us each on the tail; a transposing
                    # access pattern degrades to 144B packets)
                    nc.scalar.dma_start(out=outt[:, :, :], in_=oT[:])
                bi += 1
    nc.compile()
    return nc


def _route(x_flat: np.ndarray, router_w: np.ndarray):
    """Host-side replica of the reference router: top-2 + renormalized weights."""
    logits = x_flat @ router_w.T  # [L, E]
    m = logits.max(axis=-1, keepdims=True)
    p = np.exp(logits - m)
    p /= p.sum(axis=-1, keepdims=True)
    order = np.argsort(-p, axis=-1)[:, :K]  # [L, K]
    pv = np.take_along_axis(p, order, axis=-1)
    pv = pv / (pv.sum(axis=-1, keepdims=True) + 1e-9)
    return order, pv


def _build_in_maps(x, router_w, w1_all, b1_all, w2_all, b2_all):
    """Shared staging: router + expert-parallel dispatch lists + per-core
    input maps. Returns (cap, in_maps, idx_lists, wgt_lists)."""
    x_flat = np.asarray(x, np.float32).reshape(-1, D_MODEL)
    order, pv = _route(x_flat, np.asarray(router_w, np.float32))
    idx_lists, wgt_lists = [], []
    for e in range(E):
        sel = np.nonzero(order == e)
        idx_lists.append(sel[0])
        wgt_lists.append(pv[sel])
    max_n = max(len(t) for t in idx_lists)
    cap = max(CAP, -(-max_n // 4) * 4)
    in_maps = []
    for e in range(E):
        toks = idx_lists[e]
        n_e = len(toks)
        xg = np.zeros((cap, D_MODEL), np.float32)
        xg[:n_e] = x_flat[toks]
        w2t = np.ascontiguousarray(np.asarray(w2_all, np.float32)[e].T)
        w2merged = (
            w2t.reshape(4, 4, 128, D_MODEL)
            .transpose(0, 2, 1, 3)
            .reshape(D_MODEL, DFF)
        )
        in_maps.append(
            {
                "xgT": np.ascontiguousarray(xg.T).astype(NPBF),
                "w1t": np.ascontiguousarray(
                    np.asarray(w1_all, np.float32)[e].T
                ).astype(NPBF),
                "w2m": np.ascontiguousarray(w2merged).astype(NPBF),
                "b1r": np.ascontiguousarray(
                    np.asarray(b1_all, np.float32)[e].reshape(DFF // 128, 128, 1)
                ),
            }
        )
    return cap, in_maps, idx_lists, wgt_lists


def _get_program(cap: int):
    if cap not in _PROG_CACHE:
        _PROG_CACHE[cap] = build_program(cap)
    return _PROG_CACHE[cap]


def kernel(x, router_w, w1_all, b1_all, w2_all, b2_all):
    from concourse.bass_utils import run_bass_kernel_spmd

    x = np.asarray(x, dtype=np.float32)
    Bb, Nn, C = x.shape

    cap, in_maps, idx_lists, wgt_lists = _build_in_maps(
        x, router_w, w1_all, b1_all, w2_all, b2_all
    )
    nc = _get_program(cap)

    res = run_bass_kernel_spmd(nc, in_maps, core_ids=list(range(N_CORES)))

    # Unshard: weighted all-to-all return == scatter-add contributions per
    # token, with the expert bias b2 and combine weight folded in here.
    b2f = np.asarray(b2_all, np.float32)
    final = np.zeros((Bb * Nn, C), np.float32)
    for e in range(E):
        toks = idx_lists[e]
        ws = wgt_lists[e][:, None]
        o = res.results[e]["out"]
        if "outt" in res.results[e]:
            ot = res.results[e]["outt"]  # [128, 4, rem]; d = dj*128 + p
            o_tail = np.transpose(ot, (2, 1, 0)).reshape(ot.shape[2], -1)
            o = np.concatenate([o, o_tail], axis=0)
        o = o[: len(toks)].astype(np.float32)
        final[toks] += (o + b2f[e]) * ws
    return final.reshape(Bb, Nn, C)


def time_kernel(x, router_w, w1_all, b1_all, w2_all, b2_all, iters: int = 50):
    """Wall-clock the NEFF execution: jit once, device-put inputs, run a
    pipelined loop. Returns estimated ns per execution (all 8 cores)."""
    import time as _time

    import jax
    from jax.experimental.shard_map import shard_map
    from jax.sharding import Mesh, NamedSharding, PartitionSpec

    from concourse import bass2jax

    cap, in_maps, _, _ = _build_in_maps(x, router_w, w1_all, b1_all, w2_all, b2_all)
    nc = _get_program(cap)

    bass2jax.install_neuronx_cc_hook()

    import concourse.mybir as _mb

    partition_name = nc.partition_id_tensor.name if nc.partition_id_tensor else None
    in_names, out_names, out_avals, zero_outs = [], [], [], []
    for alloc in nc.m.functions[0].allocations:
        if not isinstance(alloc, _mb.MemoryLocationSet):
            continue
        name = alloc.memorylocations[0].name
        if alloc.kind == "ExternalInput":
            if name != partition_name:
                in_names.append(name)
        elif alloc.kind == "ExternalOutput":
            shape = tuple(alloc.tensor_shape)
            dtype = _mb.dt.np(alloc.dtype)
            out_names.append(name)
            out_avals.append(jax.core.ShapedArray(shape, dtype))
            zero_outs.append(np.zeros(shape, dtype))
    n_params = len(in_names)
    all_in_names = list(in_names) + list(out_names)
    if partition_name is not None:
        all_in_names.append(partition_name)
    if nc.dbg_addr is not None:
        extra_dbg = {nc.dbg_addr.name: np.zeros((1, 2), np.uint32)}
        in_maps = [{**m, **extra_dbg} for m in in_maps]

    def _body(*args):
        operands = list(args)
        if partition_name is not None:
            operands.append(bass2jax.partition_id_tensor())
        outs = bass2jax._bass_exec_p.bind(
            *operands,
            out_avals=tuple(out_avals),
            in_names=tuple(all_in_names),
            out_names=tuple(out_names),
            lowering_input_output_aliases=(),
            sim_require_finite=True,
            sim_require_nnan=True,
            nc=nc,
        )
        return tuple(outs)

    devices = jax.devices()[:N_CORES]
    mesh = Mesh(np.asarray(devices), ("core",))
    spec = PartitionSpec("core")
    in_specs = (spec,) * (n_params + len(out_names))
    out_specs = (spec,) * len(out_names)
    fn = jax.jit(
        shard_map(_body, mesh=mesh, in_specs=in_specs, out_specs=out_specs,
                  check_rep=False),
        keep_unused=True,
    )
    sharding = NamedSharding(mesh, spec)
    concat_in = [
        jax.device_put(
            np.concatenate([np.asarray(in_maps[c][n]) for c in range(N_CORES)], axis=0),
            sharding,
        )
        for n in in_names[:n_params]
    ]
    concat_zeros = [
        jax.device_put(
            np.zeros((N_CORES * z.shape[0], *z.shape[1:]), z.dtype), sharding
        )
        for z in zero_outs
    ]
    # warmup + compile
    outs = fn(*concat_in, *concat_zeros)
    jax.block_until_ready(outs)

    t0 = _time.perf_counter()
    for _ in range(iters):
        outs = fn(*concat_in, *concat_zeros)
    jax.block_until_ready(outs)
    dt = _time.perf_counter() - t0
    return dt / iters * 1e9


# revision 42
# speedup vs baseline: 1.0381x; 1.0381x over previous
"""MoE layer (E=8 experts, top-2 routing) on 8 Trainium2 NeuronCores.

Expert-parallel sharding: core e holds expert e's weights (w1/w2/b1).
Tokens are dispatched (host-side router, exact fp32) to the cores of their
top-2 experts; each core runs its expert's FFN on its gathered tokens in
bf16 (h = relu(x@w1+b1); out = h@w2). The bias b2 and the combine weights
are folded into the host-side scatter-add return, keeping the device
critical path to pure matmul work.

Shapes (hardcoded per the problem spec):
  x [2, 2048, 512] f32, router_w [8, 512], w1_all [8, 2048, 512],
  b1_all [8, 2048], w2_all [8, 512, 2048], b2_all [8, 512].

Per-core device program (bf16 operands, fp32 PSUM):
  mm1: h[dff, tok] = relu(w1.T-tiles @ xgT + b1)   (4 k-steps of 128)
  mm2: out[tok, d] = h-tiles.T @ w2-tiles          (16 m-steps of 128)
PE floor = cap*128 cycles = 61.4us at cap=1152; everything else (DMA
streaming, relu on DVE, PSUM->SBUF copies, output DMA) overlaps. A PE
warmup burst on a memset tile runs during the DMA head so the HAM cold
throttle (1.2GHz until ~3.5us of sustained PE activity) lifts before the
real matmuls begin.
"""

import sys

sys.path.insert(0, "/opt/trn_rl_repo")

import numpy as np
import ml_dtypes

import concourse.mybir as mybir
import concourse.tile as tile
from concourse import bacc

D_MODEL = 512
DFF = 2048
E = 8
K = 2
L = 2 * 2048  # total tokens
N_CORES = 8

FP = mybir.dt.float32
BF = mybir.dt.bfloat16
NPBF = ml_dtypes.bfloat16

# Per-expert token capacity (padded). Expected load is L*K/E = 1024 with
# std ~30 under the near-uniform router; seed-0 max count is 1092. The
# program is built for the actual max count (rounded to a multiple of 4),
# so this is only the floor. PE cost is cap*128 cycles, so every padded
# column costs 53ns — no rounding to 128.
CAP = 1092

KD = D_MODEL // 128  # 4 k-slices for mm1
MD = DFF // 128  # 16 dff tiles

_PROG_CACHE: dict = {}


def build_program(cap: int, warm: int = 9, bs: int = 512):
    """One SPMD program, run on all 8 cores; per-core data selects the expert.

    Per-core inputs (bf16 except b1):
      xgT  [512, cap]   gathered tokens for this expert, transposed (d-major)
      w1t  [512, 2048]  w1_e.T
      w2m  [512, 2048]  w2_e.T tiles merged: row J*128+p, col jj*512+d =
                        w2_e.T[(J*4+jj)*128+p, d]
      b1r  [16, 128, 1] b1_e (fp32)
    Output:
      out  [cap, 512]   bf16 expert FFN outputs (no bias/scale), row s =
                        gathered token slot s
    """
    nc = bacc.Bacc("TRN2", target_bir_lowering=False, debug=False)

    xgT = nc.dram_tensor("xgT", [D_MODEL, cap], BF, kind="ExternalInput")
    w1t = nc.dram_tensor("w1t", [D_MODEL, DFF], BF, kind="ExternalInput")
    w2m = nc.dram_tensor("w2m", [D_MODEL, DFF], BF, kind="ExternalInput")
    b1r = nc.dram_tensor("b1r", [DFF // 128, 128, 1], FP, kind="ExternalInput")
    # Full 128-token tiles go to `out` rows; a sub-128 remainder tile is
    # computed transposed (moving dim = remainder, 4x cheaper for 68 toks)
    # and lands in `outt` [d, r].
    full_cols = (cap // 128) * 128
    rem = cap - full_cols
    out = nc.dram_tensor("out", [full_cols, D_MODEL], BF, kind="ExternalOutput")
    # outt keeps the SBUF staging layout [p, dj, tok] so the final DMA is
    # one contiguous transfer; the host untangles it (d = dj*128+p).
    outt = (
        nc.dram_tensor("outt", [128, KD, rem], BF, kind="ExternalOutput")
        if rem
        else None
    )

    # token blocks: big early blocks so w1 streaming keeps up with the
    # m-loop's weight consumption; small last block to shrink the tail.
    blocks = []
    off = 0
    while off < cap:
        w = min(bs, cap - off)
        blocks.append((off, w))
        off += w

    with tile.TileContext(nc) as tc:
        with (
            tc.tile_pool(name="weights", bufs=1) as wpool,
            tc.tile_pool(name="h", bufs=4) as hpool,
            tc.tile_pool(name="psum", bufs=4, space="PSUM") as ppool,
            tc.tile_pool(name="outp", bufs=3) as opool,
            tc.tile_pool(name="consts", bufs=1) as cpool,
        ):
            xgT_sb = [
                wpool.tile([128, cap], BF, tag=f"xgT{k}", name=f"xgT_sb{k}")
                for k in range(KD)
            ]
            w1t_sb = [
                wpool.tile([128, DFF], BF, tag=f"w1t{k}", name=f"w1t_sb{k}")
                for k in range(KD)
            ]
            w2m_sb = [
                wpool.tile([128, DFF], BF, tag=f"w2m{j}", name=f"w2m_sb{j}")
                for j in range(KD)
            ]
            b1_sb = wpool.tile([128, MD], FP, tag="b1")
            warm_sb = cpool.tile([128, 512], BF, tag="warm")

            # --- DMA issue schedule ---
            # Only scalar (ACT), sync (SP) and gpsimd can issue DMAs; each
            # engine feeds its own hardware queue (~0.6us per issue, queues
            # ramp to only ~100-150 GB/s each early on). w1 is the early
            # critical stream: mm1's m-loop eats 128KB per 853ns at full
            # speed, so w1 gets strict priority and fine chunks. gpsimd
            # drops out after the w1 waves (it runs half the relu stream);
            # sync is the pure-DMA mule; scalar finishes early then owns
            # the output DMAs.
            nc.gpsimd.memset(warm_sb[:], 0.0)
            b0 = blocks[0][1]

            def xgT_dma(eng, k, lo, hi):
                eng.dma_start(
                    out=xgT_sb[k][:, lo:hi], in_=xgT[k * 128 : (k + 1) * 128, lo:hi]
                )

            def w1_dma(eng, k, lo, hi):
                eng.dma_start(
                    out=w1t_sb[k][:, lo:hi], in_=w1t[k * 128 : (k + 1) * 128, lo:hi]
                )

            # All input transfers in consumption order, strict round-robin
            # across the three issue queues so each queue's byte stream is
            # balanced and roughly need-ordered. scalar's queue head also
            # carries the ACT table load (~1.3us, for the relu stream), so
            # the rotation starts on sync/gpsimd.
            rr = [nc.sync, nc.gpsimd, nc.scalar]
            ri = 0

            def issue(dst, src):
                nonlocal ri
                rr[ri % 3].dma_start(out=dst, in_=src)
                ri += 1

            # Front order = need order, with descriptor COUNT kept minimal
            # on the early queues (each issue costs ~0.62us of queue time;
            # a finer 256-col split once regressed 12us by delaying w1).
            # The interleaved mm2 partials consume w2 m-tiles from ~2.5us
            # after the first real matmul, so w2 chunks ride between the
            # w1 waves in strict need order.
            def w2_dma(j, lo, hi):
                issue(
                    w2m_sb[j][:, lo:hi], w2m[j * 128 : (j + 1) * 128, lo:hi]
                )

            for k in range(KD):
                issue(xgT_sb[k][:, :512], xgT[k * 128 : (k + 1) * 128, :512])
            for k in range(KD):
                issue(w1t_sb[k][:, :128], w1t[k * 128 : (k + 1) * 128, :128])
            # block-1 tokens early: mm1 interleaves blocks 0/1 so each w1
            # m-tile serves two m-groups back-to-back (halves w1 demand)
            for k in range(KD):
                issue(
                    xgT_sb[k][:, 512:1024], xgT[k * 128 : (k + 1) * 128, 512:1024]
                )
            w2_dma(0, 0, 1024)  # m0-1
            for k in range(KD):
                issue(w1t_sb[k][:, 128:384], w1t[k * 128 : (k + 1) * 128, 128:384])
            nc.gpsimd.dma_start(out=b1_sb[:], in_=b1r.rearrange("m p o -> p (m o)"))
            w2_dma(0, 1024, 2048)  # m2-3
            for k in range(KD):
                issue(w1t_sb[k][:, 384:1024], w1t[k * 128 : (k + 1) * 128, 384:1024])
            w2_dma(1, 0, 1024)  # m4-5
            w2_dma(1, 1024, 2048)  # m6-7
            for k in range(KD):
                issue(
                    w1t_sb[k][:, 1024:2048], w1t[k * 128 : (k + 1) * 128, 1024:2048]
                )
            w2_dma(2, 0, 2048)  # m8-11
            w2_dma(3, 0, 2048)  # m12-15
            for k in range(KD):
                issue(
                    xgT_sb[k][:, 1024:cap], xgT[k * 128 : (k + 1) * 128, 1024:cap]
                )

            # --- PE warmup: lift the HAM cold throttle during the DMA head ---
            if warm:
                ps_w = ppool.tile([128, 512], FP, tag="ps2", bufs=4, name="ps_warm")
                for wi in range(warm):
                    nc.tensor.matmul(
                        ps_w[:], warm_sb[:, :128], warm_sb[:],
                        start=True, stop=True,
                    )
                warm_sink = cpool.tile([1, 8], FP, tag="warm_sink")
                nc.vector.tensor_copy(warm_sink[:], ps_w[0:1, 0:8])

            # --- main compute ---
            h_sb = {}

            def emit_mm1(bi, off, ncols, m):
                """mm1 m-group: ps = w1[:,m] block-matmul, then relu+bias
                into a bf16 h tile. relu split DVE/ACT (Pool can't read
                PSUM; a 512-col tile costs ~750ns on DVE / ~500ns on ACT
                vs mm1's 853ns per-m period)."""
                ps = ppool.tile([128, ncols], FP, tag="ps1", bufs=3, name="ps")
                for k in range(KD):
                    nc.tensor.matmul(
                        ps[:],
                        w1t_sb[k][:, m * 128 : (m + 1) * 128],
                        xgT_sb[k][:, off : off + ncols],
                        start=(k == 0),
                        stop=(k == KD - 1),
                    )
                h = hpool.tile([128, ncols], BF, tag=f"h{m}", name=f"h_{m}")
                # block 0's relus feed the interleaved mm2 partials, so
                # they get the dedicated DVE stream; other blocks use ACT
                # (with its ~1.3us one-time table load at queue head).
                if bi != 0:
                    nc.scalar.activation(
                        h[:],
                        ps[:],
                        mybir.ActivationFunctionType.Relu,
                        bias=b1_sb[:, m : m + 1],
                    )
                else:
                    nc.vector.tensor_scalar(
                        h[:],
                        ps[:],
                        b1_sb[:, m : m + 1],
                        0.0,
                        mybir.AluOpType.add,
                        mybir.AluOpType.max,
                    )
                h_sb[(bi, m)] = h

            def emit_mm2(bi, off, t, d_split=False):
                """mm2 for one 128-token tile + bf16 cast + output DMA.

                d_split runs two 256-wide PSUM groups (same total PE cost;
                128-col pacing shows LDWEIGHTS fully overlaps) so the first
                half's cast+DMA overlaps the second half's matmuls — used
                for the very last tile to shorten the tail."""
                u = off // 128 + t
                halves = ((0, 256), (256, 512)) if d_split else ((0, 512),)
                o = opool.tile([128, D_MODEL], BF, tag="o")
                for dlo, dhi in halves:
                    ps2 = ppool.tile(
                        [128, D_MODEL], FP, tag="ps2", bufs=4, name="ps2"
                    )
                    for m in range(MD):
                        nc.tensor.matmul(
                            ps2[:, : dhi - dlo],
                            h_sb[(bi, m)][:, t * 128 : (t + 1) * 128],
                            w2m_sb[m // 4][
                                :, (m % 4) * 512 + dlo : (m % 4) * 512 + dhi
                            ],
                            start=(m == 0),
                            stop=(m == MD - 1),
                        )
                    nc.vector.tensor_copy(o[:, dlo:dhi], ps2[:, : dhi - dlo])
                    nc.scalar.dma_start(
                        out=out[u * 128 : (u + 1) * 128, dlo:dhi],
                        in_=o[:, dlo:dhi],
                    )

            # Phase A: mm1 interleaved over blocks 0/1 with block 0's mm2
            # accumulated in open PSUM groups one m-step behind. Each
            # m-step is mm1(b0)+mm1(b1)+4 mm2 partials = 2.56us while
            # consuming one w1 m-tile (50GB/s) and one w2 m-tile — the
            # DMA queues ramp comfortably ahead of this, which kills the
            # mid-phase stalls that reset the HAM clock.
            ps2_open = [
                ppool.tile([128, D_MODEL], FP, tag="ps2", bufs=4, name=f"ps2o{t}")
                for t in range(4)
            ]
            for m in range(MD + 1):
                if m < MD:
                    emit_mm1(0, 0, 512, m)
                    emit_mm1(1, 512, 512, m)
                if m > 0:
                    pm = m - 1
                    for t in range(4):
                        nc.tensor.matmul(
                            ps2_open[t][:],
                            h_sb[(0, pm)][:, t * 128 : (t + 1) * 128],
                            w2m_sb[pm // 4][
                                :, (pm % 4) * 512 : (pm % 4 + 1) * 512
                            ],
                            start=(pm == 0),
                            stop=(pm == MD - 1),
                        )
            for t in range(4):
                o = opool.tile([128, D_MODEL], BF, tag="o")
                nc.vector.tensor_copy(o[:], ps2_open[t][:])
                nc.scalar.dma_start(out=out[t * 128 : (t + 1) * 128, :], in_=o[:])
            # Phase B: mm2 over block 1 (w2 is fully resident by now).
            for t in range(4):
                emit_mm2(1, 512, t)
            # Phase C: the tail block(s) past the first 1024 tokens. The
            # final sub-128 remainder tile (if any) runs mm2 transposed:
            # out[d, tok] with the token remainder as the moving dim —
            # 68 tokens cost 4x16x68 cycles instead of 16x512 — and one
            # small rearranged DMA. Otherwise the last full tile's mm2 is
            # d-split to shorten the tail chain.
            bi = 4
            tail_blocks = list(range(1024, cap, 512))
            for off in tail_blocks:
                w = min(512, cap - off)
                for m in range(MD):
                    emit_mm1(bi, off, w, m)
                full_t = w // 128
                for t in range(full_t):
                    last = off == tail_blocks[-1] and t == full_t - 1 and not (
                        off + w == cap and cap % 128
                    )
                    emit_mm2(bi, off, t, d_split=last)
                r = w % 128
                if off + w == cap and r:
                    oT = opool.tile([128, KD, r], BF, tag="oT", bufs=1)
                    for dj in range(KD):
                        psT = ppool.tile(
                            [128, r], FP, tag="ps2", bufs=4, name="psT"
                        )
                        for m in range(MD):
                            nc.tensor.matmul(
                                psT[:],
                                w2m_sb[m // 4][
                                    :,
                                    (m % 4) * 512
                                    + dj * 128 : (m % 4) * 512
                                    + (dj + 1) * 128,
                                ],
                                h_sb[(bi, m)][:, full_t * 128 : full_t * 128 + r],
                                start=(m == 0),
                                stop=(m == MD - 1),
                            )
                        nc.vector.tensor_copy(oT[:, dj, :], psT[:])
                    # one contiguous DMA for all four d-tiles (separate
                    # issues cost ~0.6us each on the tail; a transposing
                    # access pattern degrades to 144B packets)
                    nc.scalar.dma_start(out=outt[:, :, :], in_=oT[:])
                bi += 1
    nc.compile()
    return nc


def _route(x_flat: np.ndarray, router_w: np.ndarray):
    """Host-side replica of the reference router: top-2 + renormalized weights."""
    logits = x_flat @ router_w.T  # [L, E]
    m = logits.max(axis=-1, keepdims=True)
    p = np.exp(logits - m)
    p /= p.sum(axis=-1, keepdims=True)
    order = np.argsort(-p, axis=-1)[:, :K]  # [L, K]
    pv = np.take_along_axis(p, order, axis=-1)
    pv = pv / (pv.sum(axis=-1, keepdims=True) + 1e-9)
    return order, pv


def _build_in_maps(x, router_w, w1_all, b1_all, w2_all, b2_all):
    """Shared staging: router + expert-parallel dispatch lists + per-core
    input maps. Returns (cap, in_maps, idx_lists, wgt_lists)."""
    x_flat = np.asarray(x, np.float32).reshape(-1, D_MODEL)
    order, pv = _route(x_flat, np.asarray(router_w, np.float32))
    idx_lists, wgt_lists = [], []
    for e in range(E):
        sel = np.nonzero(order == e)
        idx_lists.append(sel[0])
        wgt_lists.append(pv[sel])
    max_n = max(len(t) for t in idx_lists)
    cap = max(CAP, -(-max_n // 4) * 4)
    in_maps = []
    for e in range(E):
        toks = idx_lists[e]
        n_e = len(toks)
        xg = np.zeros((cap, D_MODEL), np.float32)
        xg[:n_e] = x_flat[toks]
        w2t = np.ascontiguousarray(np.asarray(w2_all, np.float32)[e].T)
        w2merged = (
            w2t.reshape(4, 4, 128, D_MODEL)
            .transpose(0, 2, 1, 3)
            .reshape(D_MODEL, DFF)
        )
        in_maps.append(
            {
                "xgT": np.ascontiguousarray(xg.T).astype(NPBF),
                "w1t": np.ascontiguousarray(
                    np.asarray(w1_all, np.float32)[e].T
                ).astype(NPBF),
                "w2m": np.ascontiguousarray(w2merged).astype(NPBF),
                "b1r": np.ascontiguousarray(
                    np.asarray(b1_all, np.float32)[e].reshape(DFF // 128, 128, 1)
                ),
            }
        )
    return cap, in_maps, idx_lists, wgt_lists


def _get_program(cap: int):
    if cap not in _PROG_CACHE:
        _PROG_CACHE[cap] = build_program(cap)
    return _PROG_CACHE[cap]


def kernel(x, router_w, w1_all, b1_all, w2_all, b2_all):
    from concourse.bass_utils import run_bass_kernel_spmd

    x = np.asarray(x, dtype=np.float32)
    Bb, Nn, C = x.shape

    cap, in_maps, idx_lists, wgt_lists = _build_in_maps(
        x, router_w, w1_all, b1_all, w2_all, b2_all
    )
    nc = _get_program(cap)

    res = run_bass_kernel_spmd(nc, in_maps, core_ids=list(range(N_CORES)))

    # Unshard: weighted all-to-all return == scatter-add contributions per
    # token, with the expert bias b2 and combine weight folded in here.
    b2f = np.asarray(b2_all, np.float32)
    final = np.zeros((Bb * Nn, C), np.float32)
    for e in range(E):
        toks = idx_lists[e]
        ws = wgt_lists[e][:, None]
        o = res.results[e]["out"]
        if "outt" in res.results[e]:
            ot = res.results[e]["outt"]  # [128, 4, rem]; d = dj*128 + p
            o_tail = np.transpose(ot, (2, 1, 0)).reshape(ot.shape[2], -1)
            o = np.concatenate([o, o_tail], axis=0)
        o = o[: len(toks)].astype(np.float32)
        final[toks] += (o + b2f[e]) * ws
    return final.reshape(Bb, Nn, C)


def time_kernel(x, router_w, w1_all, b1_all, w2_all, b2_all, iters: int = 50):
    """Wall-clock the NEFF execution: jit once, device-put inputs, run a
    pipelined loop. Returns estimated ns per execution (all 8 cores)."""
    import time as _time

    import jax
    from jax.experimental.shard_map import shard_map
    from jax.sharding import Mesh, NamedSharding, PartitionSpec

    from concourse import bass2jax

    cap, in_maps, _, _ = _build_in_maps(x, router_w, w1_all, b1_all, w2_all, b2_all)
    nc = _get_program(cap)

    bass2jax.install_neuronx_cc_hook()

    import concourse.mybir as _mb

    partition_name = nc.partition_id_tensor.name if nc.partition_id_tensor else None
    in_names, out_names, out_avals, zero_outs = [], [], [], []
    for alloc in nc.m.functions[0].allocations:
        if not isinstance(alloc, _mb.MemoryLocationSet):
            continue
        name = alloc.memorylocations[0].name
        if alloc.kind == "ExternalInput":
            if name != partition_name:
                in_names.append(name)
        elif alloc.kind == "ExternalOutput":
            shape = tuple(alloc.tensor_shape)
            dtype = _mb.dt.np(alloc.dtype)
            out_names.append(name)
            out_avals.append(jax.core.ShapedArray(shape, dtype))
            zero_outs.append(np.zeros(shape, dtype))
    n_params = len(in_names)
    all_in_names = list(in_names) + list(out_names)
    if partition_name is not None:
        all_in_names.append(partition_name)
    if nc.dbg_addr is not None:
        extra_dbg = {nc.dbg_addr.name: np.zeros((1, 2), np.uint32)}
        in_maps = [{**m, **extra_dbg} for m in in_maps]

    def _body(*args):
        operands = list(args)
        if partition_name is not None:
            operands.append(bass2jax.partition_id_tensor())
        outs = bass2jax._bass_exec_p.bind(
            *operands,
            out_avals=tuple(out_avals),
            in_names=tuple(all_in_names),
            out_names=tuple(out_names),
            lowering_input_output_aliases=(),
            sim_require_finite=True,
            sim_require_nnan=True,
            nc=nc,
        )
        return tuple(outs)

    devices = jax.devices()[:N_CORES]
    mesh = Mesh(np.asarray(devices), ("core",))
    spec = PartitionSpec("core")
    in_specs = (spec,) * (n_params + len(out_names))
    out_specs = (spec,) * len(out_names)
    fn = jax.jit(
        shard_map(_body, mesh=mesh, in_specs=in_specs, out_specs=out_specs,
                  check_rep=False),
        keep_unused=True,
    )
    sharding = NamedSharding(mesh, spec)
    concat_in = [
        jax.device_put(
            np.concatenate([np.asarray(in_maps[c][n]) for c in range(N_CORES)], axis=0),
            sharding,
        )
        for n in in_names[:n_params]
    ]
    concat_zeros = [
        jax.device_put(
            np.zeros((N_CORES * z.shape[0], *z.shape[1:]), z.dtype), sharding
        )
        for z in zero_outs
    ]
    # warmup + compile
    outs = fn(*concat_in, *concat_zeros)
    jax.block_until_ready(outs)

    t0 = _time.perf_counter()
    for _ in range(iters):
        outs = fn(*concat_in, *concat_zeros)
    jax.block_until_ready(outs)
    dt = _time.perf_counter() - t0
    return dt / iters * 1e9
